# revision 4
# baseline (speedup 1.0000x reference)
"""Delta-modulator scan kernel for Trainium2 — raw bass (no Tile).

Per (b, r): sequential scan over the first 232 columns of x[.,.,252] with
state (dc, delta, signed run-counter); outputs UP[232] | DN[232] | x[232:252]
-> out [., ., 484] f32. Data-parallel over batch: 16 batches/core, 8 cores;
per-core 16384 instances laid out as [128 partitions x 128 free].

Structure (all timings from the TimelineSim cost model):
- The whole scan runs on the DVE, back-to-back, with NO semaphores inside
  the loop (in-order engine; full-width streaming ops make same-engine RAW
  safe): y = x - dc; v = (y>dl)-(y<-dl) written into xv slot t (in place
  over the consumed x column); copy_predicated dc-latch; run-counter;
  delta update. 5 ops x ~194ns x 232 steps ~= 225us; Tile's per-dependency
  semaphore chains would add ~190ns/step, which is why this is raw bass.
- Input loads: [0:24) first (small, so the scan starts at ~12us), then
  [24:152) and [104:232) as 128-column transfers whose 512B contiguous
  runs avoid the <512B 2x DMA penalty. The overlap region is rewritten
  with identical data.
- up/dn are extracted from the v history by the otherwise-idle Activation
  engine (relu(v), relu(-v)) into small staging tiles, in column chunks
  sized so the store finish time max_k(ready_k + remaining store mass) is
  minimized: 32-col early, 24/20-col late, 16-col last (floor-rate pair).
  Each extraction is split (n-1)+1 so stores fire as soon as the chunk's
  last column lands.
- Tail passthrough out[464:484) = x[232:252) as a direct DRAM->DRAM DMA.
- Manual semaphores: dma_sem (+16/DMA, FIFO order), dve_sem (+1 per step
  via copy_predicated, +1 by the init memset, +1 by the final DM_V),
  act_sem (+1 per extraction piece).
"""

import os
from contextlib import ExitStack

import numpy as np

import concourse.bass as bass
from concourse import bacc, mybir
from concourse.bass_utils import run_bass_kernel_spmd
import concourse.dve_ops as dve_ops_mod
from concourse.dve_spec import (
    Spec, Src0, Src1, C0, C1, C2, Zero, One, maxx, minn, select, lower,
)
from concourse.dve_spec import _has_src1
from concourse.dve_uop import DveOpSpec

AluOp = mybir.AluOpType
F32 = mybir.dt.float32


def _register_op(name: str, spec: Spec) -> "dve_ops_mod.DveOp":
    """Register a custom DVE op at runtime (compute + pin its uop sha)."""
    for existing in dve_ops_mod.OPS:
        if existing.name == name:
            return existing
    opcode = dve_ops_mod._CUSTOM_DVE_ROW_BASE + len(dve_ops_mod.OPS)
    assert opcode < 0x20
    shas = {}
    for ver in ("v3",):
        tmp = DveOpSpec(
            name=name, opcode=opcode, uops=lower(spec, ver=ver), rd1_en=_has_src1(spec)
        )
        shas[ver] = tmp.sha(ver)
    op = dve_ops_mod.DveOp(name, spec, subdim=False, uops_sha=shas)
    dve_ops_mod.OPS.append(op)
    dve_ops_mod._SUB_OPCODE_FOR_NAME[name] = opcode
    dve_ops_mod.CUSTOM_DVE_SPECS[name] = spec
    return op


# cc' = trig ? max(cc,0)+1 : min(cc,0)-1   (in0=cc, in1=v; trig = in1 != 0)
DM_COUNTER = _register_op(
    "DM_COUNTER_ANT",
    Spec(
        body=select(Src1, maxx(Src0, Zero) + One, minn(Src0, Zero) - One),
        reference=lambda in0, in1, s0, s1, imm2: np.where(
            in1 != 0.0, np.maximum(in0, 0) + 1, np.minimum(in0, 0) - 1
        ).astype(np.float32),
    ),
)

# dl' = min(max(dl, (cc<=-3)*0.1), max((cc<3), 0.02))  (in0=cc, in1=dl,
# s0=-3.0, s1=0.1, imm2=0.02)
DM_DELTA = _register_op(
    "DM_DELTA_ANT",
    Spec(
        body=minn(
            maxx(Src1, (Src0 <= C0) * C1),
            maxx(Src0 < (Zero - C0), C2),
        ),
        reference=lambda in0, in1, s0, s1, imm2: np.minimum(
            np.maximum(in1, (in0 <= s0).astype(np.float32) * s1),
            np.maximum((in0 < -s0).astype(np.float32), imm2),
        ).astype(np.float32),
    ),
)

# v = (y > dl) - (y < -dl) in {-1, 0, +1} (never -0.0). Doubles as the
# predication mask (bit pattern nonzero iff trigger).  (in0=y, in1=dl)
DM_V = _register_op(
    "DM_V2_ANT",
    Spec(
        body=(Src0 > Src1) - (Src0 < (Zero - Src1)),
        reference=lambda in0, in1, s0, s1, imm2: (
            (in0 > in1).astype(np.float32) - (in0 < -in1).astype(np.float32)
        ),
    ),
)

B, R, C = 128, 1024, 252
NSTEP = 232
NTAIL = C - NSTEP  # 20
OUTC = 2 * NSTEP + NTAIL  # 484
NCORES = 8
BPC = B // NCORES  # 16
INST = BPC * R  # 16384 instances per core
P = 128
F = INST // P  # 128

# store chunking: 32-col early, then 24/20-col late chunks. 20-col pairs
# hit the 7ns/descriptor floor at the same per-column rate as 32-col but
# carry less mass after the scan's last step, which bounds the finish.
CHUNKS = [(0, 32), (32, 32), (64, 32), (96, 32), (128, 24), (152, 24),
          (176, 20), (196, 20), (216, 16)]

_NC_CACHE = {}


def _build_nc() -> bass.Bass:
    key = "nc"
    if key in _NC_CACHE:
        return _NC_CACHE[key]
    nc = bacc.Bacc("TRN2", target_bir_lowering=False, debug=False)
    x = nc.dram_tensor("x", [INST, C], F32, kind="ExternalInput").ap()
    out = nc.dram_tensor("out", [INST, OUTC], F32, kind="ExternalOutput").ap()
    x3 = x.rearrange("(p f) c -> p f c", p=P)  # [128, 128, 252]
    o3 = out.rearrange("(p f) c -> p f c", p=P)  # [128, 128, 484]

    Relu = mybir.ActivationFunctionType.Relu
    NDMA = 4 + 2 * len(CHUNKS)

    with ExitStack() as ctx:
        # xv slot k holds x_{k-1}; v_t overwrites slot t (x_{t-1} dead).
        xv_t = ctx.enter_context(nc.sbuf_tensor("xv", [P, F, NSTEP + 1], F32))
        s_up_t = ctx.enter_context(nc.sbuf_tensor("s_up", [P, F, 32], F32))
        s_dn_t = ctx.enter_context(nc.sbuf_tensor("s_dn", [P, F, 32], F32))
        dc_t = ctx.enter_context(nc.sbuf_tensor("dc", [P, F], F32))
        dl_ts = [
            ctx.enter_context(nc.sbuf_tensor(f"dl{i}", [P, F], F32))
            for i in range(2)
        ]
        cc_ts = [
            ctx.enter_context(nc.sbuf_tensor(f"cc{i}", [P, F], F32))
            for i in range(2)
        ]
        y_ts = [
            ctx.enter_context(nc.sbuf_tensor(f"y{i}", [P, F], F32))
            for i in range(2)
        ]
        dma_sem = ctx.enter_context(nc.semaphore("dma_sem"))
        dve_sem = ctx.enter_context(nc.semaphore("dve_sem"))
        act_sem = ctx.enter_context(nc.semaphore("act_sem"))

        xv = xv_t.ap()
        s_up = s_up_t.ap()
        s_dn = s_dn_t.ap()
        dc = dc_t.ap()
        dls = [t.ap() for t in dl_ts]
        ccs = [t.ap() for t in cc_ts]
        ys = [t.ap() for t in y_ts]

        with nc.Block() as block:

            @block.sync
            def _(sync_):
                sync = sync_
                # loads: x_k -> slot k+1
                sync.dma_start(xv[:, :, 1:25], x3[:, :, 0:24]).then_inc(
                    dma_sem, 16
                )
                sync.dma_start(xv[:, :, 25:153], x3[:, :, 24:152]).then_inc(
                    dma_sem, 16
                )
                sync.dma_start(xv[:, :, 105:233], x3[:, :, 104:232]).then_inc(
                    dma_sem, 16
                )
                # tail passthrough DRAM->DRAM
                sync.dma_start(
                    o3[:, :, 2 * NSTEP : OUTC], x3[:, :, NSTEP:C]
                ).then_inc(dma_sem, 16)
                for k, (c0, cn) in enumerate(CHUNKS):
                    sync.wait_ge(act_sem, 4 * k + 2)
                    sync.dma_start(
                        o3[:, :, c0 : c0 + cn], s_up[:, :, 0:cn]
                    ).then_inc(dma_sem, 16)
                    sync.wait_ge(act_sem, 4 * k + 4)
                    sync.dma_start(
                        o3[:, :, NSTEP + c0 : NSTEP + c0 + cn], s_dn[:, :, 0:cn]
                    ).then_inc(dma_sem, 16)
                sync.wait_ge(dma_sem, 16 * NDMA)

            @block.vector
            def _(vector):
                vector.memset(dc[:], 0.0)
                vector.memset(dls[0][:], 0.1)
                vector.memset(ccs[0][:], 0.0).then_inc(dve_sem)
                dli = cci = 0
                for t in range(NSTEP):
                    if t == 0:
                        vector.wait_ge(dma_sem, 16)
                    elif t == 24:
                        vector.wait_ge(dma_sem, 32)
                    elif t == 105:
                        vector.wait_ge(dma_sem, 48)
                    xs = xv[:, :, t + 1]
                    y = ys[t % 2]
                    dl, cc = dls[dli], ccs[cci]
                    dl2, cc2 = dls[1 - dli], ccs[1 - cci]
                    vslot = xv[:, :, t]
                    vector.tensor_tensor(y[:], xs, dc[:], AluOp.subtract)
                    if t == NSTEP - 1:
                        # final step: only v is consumed (by extraction)
                        vector._custom_dve(
                            DM_V, out=vslot, in0=y[:], in1=dl[:]
                        ).then_inc(dve_sem)
                        break
                    vector._custom_dve(DM_V, out=vslot, in0=y[:], in1=dl[:])
                    vector.copy_predicated(
                        dc[:], vslot.bitcast(mybir.dt.int32), xs
                    ).then_inc(dve_sem)
                    vector._custom_dve(
                        DM_COUNTER, out=cc2[:], in0=cc[:], in1=vslot
                    )
                    vector._custom_dve(
                        DM_DELTA, out=dl2[:], in0=cc2[:], in1=dl[:],
                        s0=-3.0, s1=0.1, imm2=0.02,
                    )
                    dli, cci = 1 - dli, 1 - cci

            @block.scalar
            def _(scalar):
                # each plane's extraction is split (cn-1)+1 so the store can
                # fire as soon as the chunk's LAST column's v lands.
                for k, (c0, cn) in enumerate(CHUNKS):
                    scalar.wait_ge(dve_sem, c0 + cn)  # v through col c0+cn-2
                    if k >= 1:
                        # stage WAR: previous up-store (DMA #(2k+3)) done
                        scalar.wait_ge(dma_sem, 16 * (2 * k + 3))
                    scalar.activation(
                        s_up[:, :, 0 : cn - 1],
                        xv[:, :, c0 : c0 + cn - 1], Relu, 0.0, 1.0,
                    ).then_inc(act_sem)
                    scalar.wait_ge(dve_sem, 1 + c0 + cn)  # v(c0+cn-1) done
                    scalar.activation(
                        s_up[:, :, cn - 1 : cn],
                        xv[:, :, c0 + cn - 1 : c0 + cn], Relu, 0.0, 1.0,
                    ).then_inc(act_sem)
                    if k >= 1:
                        scalar.wait_ge(dma_sem, 16 * (2 * k + 4))
                    scalar.activation(
                        s_dn[:, :, 0 : cn - 1],
                        xv[:, :, c0 : c0 + cn - 1], Relu, 0.0, -1.0,
                    ).then_inc(act_sem)
                    scalar.activation(
                        s_dn[:, :, cn - 1 : cn],
                        xv[:, :, c0 + cn - 1 : c0 + cn], Relu, 0.0, -1.0,
                    ).then_inc(act_sem)

    nc.compile()
    _NC_CACHE[key] = nc
    return nc


def kernel(x: np.ndarray) -> np.ndarray:
    x = np.ascontiguousarray(np.asarray(x), dtype=np.float32)
    assert x.shape == (B, R, C), x.shape
    nc = _build_nc()
    in_maps = [
        {"x": np.ascontiguousarray(x[c * BPC : (c + 1) * BPC].reshape(INST, C))}
        for c in range(NCORES)
    ]
    res = run_bass_kernel_spmd(
        nc,
        in_maps,
        core_ids=list(range(NCORES)),
        trace=bool(int(os.environ.get("KERNEL_TRACE", "0"))),
    )
    global LAST_RESULTS
    LAST_RESULTS = res
    outs = [r["out"].reshape(BPC, R, OUTC) for r in res.results]
    return np.concatenate(outs, axis=0)


LAST_RESULTS = None


if __name__ == "__main__":
    xs = np.random.default_rng(0).standard_normal((B, R, C), dtype=np.float32)
    o = kernel(xs)
    print(o.shape, o.dtype)


# revision 5
# speedup vs baseline: 1.0084x; 1.0084x over previous
"""Delta-modulator scan kernel for Trainium2 — V12: raw bass (no Tile).

Per (b, r): sequential scan over the first 232 columns of x[.,.,252] with
state (dc, delta, signed run-counter); outputs UP[232] | DN[232] | x[232:252]
-> out [., ., 484] f32. Data-parallel over batch: 16 batches/core, 8 cores;
per-core 16384 instances laid out as [128 partitions x 128 free].

Structure:
- All scan ops on the DVE, back-to-back (in-order engine, no semaphores
  inside the loop): y = x - dc; v = (y>dl)-(y<-dl) -> xv slot t (in-place
  over the consumed x column); copy_predicated dc; counter; delta.
- Input loads: [0:32) (small, so the scan starts early), [32:160) and
  [104:232) as 128-column transfers (512B contiguous runs = full DMA rate).
- up/dn extracted from the v history by the Activation engine in 32-column
  chunks (relu(v), relu(-v)) into small staging tiles; SP stores each chunk.
- Tail passthrough out[464:484) = x[232:252) as a direct DRAM->DRAM DMA.
- Manual semaphores: dma_sem (+16/DMA, FIFO), dve_sem (+1 per scan step via
  copy_predicated, +1 by the init memset), act_sem (+1 per extraction).
"""

import os
from contextlib import ExitStack

import numpy as np

import concourse.bass as bass
from concourse import bacc, mybir
from concourse.bass_utils import run_bass_kernel_spmd
import concourse.dve_ops as dve_ops_mod
from concourse.dve_spec import (
    Spec, Src0, Src1, C0, C1, C2, Zero, One, maxx, minn, select, lower,
)
from concourse.dve_spec import _has_src1
from concourse.dve_uop import DveOpSpec

AluOp = mybir.AluOpType
F32 = mybir.dt.float32


def _register_op(name: str, spec: Spec) -> "dve_ops_mod.DveOp":
    """Register a custom DVE op at runtime (compute + pin its uop sha)."""
    for existing in dve_ops_mod.OPS:
        if existing.name == name:
            return existing
    opcode = dve_ops_mod._CUSTOM_DVE_ROW_BASE + len(dve_ops_mod.OPS)
    assert opcode < 0x20
    shas = {}
    for ver in ("v3",):
        tmp = DveOpSpec(
            name=name, opcode=opcode, uops=lower(spec, ver=ver), rd1_en=_has_src1(spec)
        )
        shas[ver] = tmp.sha(ver)
    op = dve_ops_mod.DveOp(name, spec, subdim=False, uops_sha=shas)
    dve_ops_mod.OPS.append(op)
    dve_ops_mod._SUB_OPCODE_FOR_NAME[name] = opcode
    dve_ops_mod.CUSTOM_DVE_SPECS[name] = spec
    return op


# cc' = trig ? max(cc,0)+1 : min(cc,0)-1   (in0=cc, in1=v; trig = in1 != 0)
DM_COUNTER = _register_op(
    "DM_COUNTER_ANT",
    Spec(
        body=select(Src1, maxx(Src0, Zero) + One, minn(Src0, Zero) - One),
        reference=lambda in0, in1, s0, s1, imm2: np.where(
            in1 != 0.0, np.maximum(in0, 0) + 1, np.minimum(in0, 0) - 1
        ).astype(np.float32),
    ),
)

# dl' = min(max(dl, (cc<=-3)*0.1), max((cc<3), 0.02))  (in0=cc, in1=dl,
# s0=-3.0, s1=0.1, imm2=0.02)
DM_DELTA = _register_op(
    "DM_DELTA_ANT",
    Spec(
        body=minn(
            maxx(Src1, (Src0 <= C0) * C1),
            maxx(Src0 < (Zero - C0), C2),
        ),
        reference=lambda in0, in1, s0, s1, imm2: np.minimum(
            np.maximum(in1, (in0 <= s0).astype(np.float32) * s1),
            np.maximum((in0 < -s0).astype(np.float32), imm2),
        ).astype(np.float32),
    ),
)

# v = (y > dl) - (y < -dl) in {-1, 0, +1} (never -0.0). Doubles as the
# predication mask (bit pattern nonzero iff trigger).  (in0=y, in1=dl)
DM_V = _register_op(
    "DM_V2_ANT",
    Spec(
        body=(Src0 > Src1) - (Src0 < (Zero - Src1)),
        reference=lambda in0, in1, s0, s1, imm2: (
            (in0 > in1).astype(np.float32) - (in0 < -in1).astype(np.float32)
        ),
    ),
)

B, R, C = 128, 1024, 252
NSTEP = 232
NTAIL = C - NSTEP  # 20
OUTC = 2 * NSTEP + NTAIL  # 484
NCORES = 8
BPC = B // NCORES  # 16
INST = BPC * R  # 16384 instances per core
P = 128
F = INST // P  # 128

# store chunking: 32-col early, then 24/20-col late chunks. 20-col pairs
# hit the 7ns/descriptor floor at the same per-column rate as 32-col but
# carry less mass after the scan's last step, which bounds the finish.
CHUNKS = [(0, 32), (32, 32), (64, 32), (96, 32), (128, 24), (152, 24),
          (176, 20), (196, 20), (216, 16)]

_NC_CACHE = {}


def _build_nc() -> bass.Bass:
    key = "nc"
    if key in _NC_CACHE:
        return _NC_CACHE[key]
    nc = bacc.Bacc("TRN2", target_bir_lowering=False, debug=False)
    x = nc.dram_tensor("x", [INST, C], F32, kind="ExternalInput").ap()
    out = nc.dram_tensor("out", [INST, OUTC], F32, kind="ExternalOutput").ap()
    x3 = x.rearrange("(p f) c -> p f c", p=P)  # [128, 128, 252]
    o3 = out.rearrange("(p f) c -> p f c", p=P)  # [128, 128, 484]

    Relu = mybir.ActivationFunctionType.Relu
    NDMA = 5 + 2 * len(CHUNKS)

    def _pieces(k, cn):
        # split each plane's extraction so the store can fire right after
        # the chunk's LAST column's v lands; the final chunk gets a 3-way
        # split so the Act engine is idle when column 231 arrives.
        if k == len(CHUNKS) - 1:
            return [cn - 4, 3, 1]
        return [cn - 1, 1]

    with ExitStack() as ctx:
        # xv slot k holds x_{k-1}; v_t overwrites slot t (x_{t-1} dead).
        xv_t = ctx.enter_context(nc.sbuf_tensor("xv", [P, F, NSTEP + 1], F32))
        s_up_t = ctx.enter_context(nc.sbuf_tensor("s_up", [P, F, 32], F32))
        s_dn_t = ctx.enter_context(nc.sbuf_tensor("s_dn", [P, F, 32], F32))
        dc_t = ctx.enter_context(nc.sbuf_tensor("dc", [P, F], F32))
        dl_ts = [
            ctx.enter_context(nc.sbuf_tensor(f"dl{i}", [P, F], F32))
            for i in range(2)
        ]
        cc_ts = [
            ctx.enter_context(nc.sbuf_tensor(f"cc{i}", [P, F], F32))
            for i in range(2)
        ]
        y_ts = [
            ctx.enter_context(nc.sbuf_tensor(f"y{i}", [P, F], F32))
            for i in range(2)
        ]
        dma_sem = ctx.enter_context(nc.semaphore("dma_sem"))
        dve_sem = ctx.enter_context(nc.semaphore("dve_sem"))
        act_sem = ctx.enter_context(nc.semaphore("act_sem"))

        xv = xv_t.ap()
        s_up = s_up_t.ap()
        s_dn = s_dn_t.ap()
        dc = dc_t.ap()
        dls = [t.ap() for t in dl_ts]
        ccs = [t.ap() for t in cc_ts]
        ys = [t.ap() for t in y_ts]

        with nc.Block() as block:

            @block.sync
            def _(sync_):
                sync = sync_
                # loads: x_k -> slot k+1
                sync.dma_start(xv[:, :, 1:17], x3[:, :, 0:16]).then_inc(
                    dma_sem, 16
                )
                sync.dma_start(xv[:, :, 17:33], x3[:, :, 16:32]).then_inc(
                    dma_sem, 16
                )
                sync.dma_start(xv[:, :, 33:161], x3[:, :, 32:160]).then_inc(
                    dma_sem, 16
                )
                sync.dma_start(xv[:, :, 105:233], x3[:, :, 104:232]).then_inc(
                    dma_sem, 16
                )
                # tail passthrough DRAM->DRAM
                sync.dma_start(
                    o3[:, :, 2 * NSTEP : OUTC], x3[:, :, NSTEP:C]
                ).then_inc(dma_sem, 16)
                nact = 0
                for k, (c0, cn) in enumerate(CHUNKS):
                    nact += len(_pieces(k, cn))
                    sync.wait_ge(act_sem, nact)
                    sync.dma_start(
                        o3[:, :, c0 : c0 + cn], s_up[:, :, 0:cn]
                    ).then_inc(dma_sem, 16)
                    nact += len(_pieces(k, cn))
                    sync.wait_ge(act_sem, nact)
                    sync.dma_start(
                        o3[:, :, NSTEP + c0 : NSTEP + c0 + cn], s_dn[:, :, 0:cn]
                    ).then_inc(dma_sem, 16)
                sync.wait_ge(dma_sem, 16 * NDMA)

            @block.vector
            def _(vector):
                vector.memset(dc[:], 0.0)
                vector.memset(dls[0][:], 0.1)
                vector.memset(ccs[0][:], 0.0).then_inc(dve_sem)
                dli = cci = 0
                for t in range(NSTEP):
                    if t == 0:
                        vector.wait_ge(dma_sem, 16)
                    elif t == 16:
                        vector.wait_ge(dma_sem, 32)
                    elif t == 32:
                        vector.wait_ge(dma_sem, 48)
                    elif t == 105:
                        vector.wait_ge(dma_sem, 64)
                    xs = xv[:, :, t + 1]
                    y = ys[t % 2]
                    dl, cc = dls[dli], ccs[cci]
                    dl2, cc2 = dls[1 - dli], ccs[1 - cci]
                    vslot = xv[:, :, t]
                    vector.tensor_tensor(y[:], xs, dc[:], AluOp.subtract)
                    if t == NSTEP - 1:
                        # final step: only v is consumed (by extraction)
                        vector._custom_dve(
                            DM_V, out=vslot, in0=y[:], in1=dl[:]
                        ).then_inc(dve_sem)
                        break
                    vector._custom_dve(DM_V, out=vslot, in0=y[:], in1=dl[:])
                    vector.copy_predicated(
                        dc[:], vslot.bitcast(mybir.dt.int32), xs
                    ).then_inc(dve_sem)
                    vector._custom_dve(
                        DM_COUNTER, out=cc2[:], in0=cc[:], in1=vslot
                    )
                    vector._custom_dve(
                        DM_DELTA, out=dl2[:], in0=cc2[:], in1=dl[:],
                        s0=-3.0, s1=0.1, imm2=0.02,
                    )
                    dli, cci = 1 - dli, 1 - cci

            @block.scalar
            def _(scalar):
                for k, (c0, cn) in enumerate(CHUNKS):
                    pieces = _pieces(k, cn)
                    for stage, scale, base, war in (
                        (s_up, 1.0, c0, 16 * (2 * k + 4)),
                        (s_dn, -1.0, NSTEP + c0, 16 * (2 * k + 5)),
                    ):
                        p0 = 0
                        first = True
                        for pn in pieces:
                            # piece covers cols [c0+p0, c0+p0+pn): needs
                            # v(c0+p0+pn-1): dve_sem >= 1+c0+p0+pn (via
                            # cp inc; the final step's DM_V carries it)
                            scalar.wait_ge(dve_sem, 1 + c0 + p0 + pn)
                            if first and k >= 1:
                                # stage WAR: previous store of this plane
                                scalar.wait_ge(dma_sem, war)
                                first = False
                            scalar.activation(
                                stage[:, :, p0 : p0 + pn],
                                xv[:, :, c0 + p0 : c0 + p0 + pn],
                                Relu, 0.0, scale,
                            ).then_inc(act_sem)
                            p0 += pn

    nc.compile()
    _NC_CACHE[key] = nc
    return nc


def kernel(x: np.ndarray) -> np.ndarray:
    x = np.ascontiguousarray(np.asarray(x), dtype=np.float32)
    assert x.shape == (B, R, C), x.shape
    nc = _build_nc()
    in_maps = [
        {"x": np.ascontiguousarray(x[c * BPC : (c + 1) * BPC].reshape(INST, C))}
        for c in range(NCORES)
    ]
    res = run_bass_kernel_spmd(
        nc,
        in_maps,
        core_ids=list(range(NCORES)),
        trace=bool(int(os.environ.get("KERNEL_TRACE", "0"))),
    )
    global LAST_RESULTS
    LAST_RESULTS = res
    outs = [r["out"].reshape(BPC, R, OUTC) for r in res.results]
    return np.concatenate(outs, axis=0)


LAST_RESULTS = None


if __name__ == "__main__":
    xs = np.random.default_rng(0).standard_normal((B, R, C), dtype=np.float32)
    o = kernel(xs)
    print(o.shape, o.dtype)
